# revision 1
# baseline (speedup 1.0000x reference)
"""Trainium2 Bass kernel for grouped block-diagonal MLP (gnn_message_passing).

Computation: out[b, 3g+j] = sum_i x[b, 15g+i] * W[g, j, i]   (g<25, i<15, j<3)
Equivalent to out = x @ Wd where Wd is a [375, 75] block-diagonal matrix built
from the 25 stacked [3, 15] Linear weights (scattered per k_idx/v_idx).

Strategy (pure data parallel, 8 cores):
  - shard batch dim of x (262144 rows -> 8 x 32768), replicate Wd
  - per core: stream x in [128, 8, 375] super-tiles (natural layout, fully
    contiguous HBM reads), PE-transpose each [128, chunk] to get the
    contraction dim onto partitions, copy PSUM->SBUF, then 3 accumulating
    matmuls against resident Wd chunks -> out tile [128, 75] -> DMA out.
"""

import numpy as np

B = 262144
NCORES = 8
B_CORE = B // NCORES  # 32768
F = 375  # input cols  (25 groups * 15)
O = 75   # output cols (25 groups * 3)
OUT_DIM = 75  # hard-coded output width of the reference
CHUNKS = [(0, 128), (128, 128), (256, 119)]  # (offset, size) along F
SUP = 8            # 128-row tiles per super-tile (DMA batching)
ROWS_SUP = 128 * SUP
N_SUP = B_CORE // ROWS_SUP  # 32

_compiled = {}


def _build_bass():
    import concourse.bass as bass
    import concourse.mybir as mybir
    import concourse.tile as tile
    from concourse import bacc
    from concourse.masks import make_identity

    f32 = mybir.dt.float32
    nc = bacc.Bacc()
    x_d = nc.dram_tensor("x", [B_CORE, F], f32, kind="ExternalInput")
    w_d = nc.dram_tensor("wd", [3, 128, O], f32, kind="ExternalInput")
    o_d = nc.dram_tensor("out", [B_CORE, O], f32, kind="ExternalOutput")

    with tile.TileContext(nc) as tc:
        with (
            tc.tile_pool(name="const", bufs=1) as cpool,
            tc.tile_pool(name="xin", bufs=3) as xpool,
            tc.tile_pool(name="xt", bufs=6) as xtpool,
            tc.tile_pool(name="res", bufs=3) as rpool,
            tc.tile_pool(name="pst", bufs=4, space="PSUM") as pst,
            tc.tile_pool(name="acc", bufs=2, space="PSUM") as pacc,
            tc.tile_pool(name="warm", bufs=1, space="PSUM") as pwarm,
        ):
            ident = cpool.tile([128, 128], f32)
            make_identity(nc, ident[:])
            wd = cpool.tile([128, 3, O], f32)
            nc.sync.dma_start(wd[:], w_d[:].rearrange("c k n -> k c n"))

            # PE instructions carry at most one semaphore wait. Absorb the
            # identity (Pool) and wd (DMA) deps with throwaway transposes so
            # the first real transpose only waits on its x DMA.
            warm = pwarm.tile([128, 128], f32)
            nc.tensor.transpose(warm[:, :], ident[:], ident[:])
            nc.tensor.transpose(warm[:O, :], wd[:, 0, :], ident[:])

            for s in range(N_SUP):
                row0 = s * ROWS_SUP
                xin = xpool.tile([128, SUP, F], f32)
                nc.sync.dma_start(
                    xin[:],
                    x_d[row0 : row0 + ROWS_SUP, :].rearrange(
                        "(t p) f -> p t f", p=128
                    ),
                )
                outt = rpool.tile([128, SUP, O], f32)
                for t in range(SUP):
                    acc = pacc.tile([128, O], f32)
                    xts = []
                    for c, (off, sz) in enumerate(CHUNKS):
                        xt_ps = pst.tile([128, 128], f32, tag="xt_ps")
                        nc.tensor.transpose(
                            xt_ps[:sz, :], xin[:, t, off : off + sz], ident[:]
                        )
                        xt_sb = xtpool.tile([128, 128], f32, tag="xt_sb")
                        nc.vector.tensor_copy(xt_sb[:sz, :], xt_ps[:sz, :])
                        xts.append(xt_sb)
                    for c, (off, sz) in enumerate(CHUNKS):
                        nc.tensor.matmul(
                            acc[:],
                            xts[c][:sz, :],
                            wd[:sz, c, :],
                            start=(c == 0),
                            stop=(c == 2),
                        )
                    nc.vector.tensor_copy(outt[:, t, :], acc[:])
                nc.sync.dma_start(
                    o_d[row0 : row0 + ROWS_SUP, :].rearrange(
                        "(t p) f -> p t f", p=128
                    ),
                    outt[:],
                )
    nc.compile()
    return nc


def _get_nc():
    if "nc" not in _compiled:
        _compiled["nc"] = _build_bass()
    return _compiled["nc"]


def _build_wd_chunks(W, k_idx, v_idx):
    """Dense [3, 128, 75] chunked block-diagonal weight from stacked W."""
    Wd = np.zeros((384, O), dtype=np.float32)
    kk = np.asarray(k_idx)
    vv = np.asarray(v_idx)
    Ww = np.asarray(W)
    # Wd[k_idx[g,i], v_idx[g,j]] = W[g, j, i]
    Wd[kk[:, :, None], vv[:, None, :]] = Ww.transpose(0, 2, 1)
    return np.ascontiguousarray(Wd.reshape(3, 128, O))


def kernel(x, W, k_idx, v_idx, **_unused):
    from concourse.bass_utils import run_bass_kernel_spmd

    x = np.asarray(x, dtype=np.float32)
    wd3 = _build_wd_chunks(W, k_idx, v_idx)
    nc = _get_nc()

    in_maps = [
        {"x": np.ascontiguousarray(x[i * B_CORE : (i + 1) * B_CORE]), "wd": wd3}
        for i in range(NCORES)
    ]
    res = run_bass_kernel_spmd(nc, in_maps, list(range(NCORES)))
    parts = [res.results[i]["out"] for i in range(NCORES)]
    got = np.concatenate(parts, axis=0)

    vflat = np.asarray(v_idx).reshape(-1)
    if vflat.shape[0] == OUT_DIM and np.array_equal(vflat, np.arange(OUT_DIM)):
        return np.ascontiguousarray(got.astype(np.float32))
    out = np.zeros((x.shape[0], OUT_DIM), dtype=np.float32)
    out[:, vflat] = got
    return out



# revision 4
# speedup vs baseline: 2.3953x; 2.3953x over previous
"""Trainium2 Bass kernel for grouped block-diagonal MLP (gnn_message_passing).

Computation: out[b, 3g+j] = sum_i x[b, 15g+i] * W[g, j, i]   (g<25, i<15, j<3)
Equivalent to out = x @ Wd where Wd is a [375, 75] block-diagonal matrix built
from the 25 stacked [3, 15] Linear weights (scattered per k_idx/v_idx).

Strategy (pure data parallel, 8 cores), v2:
  - memory-regime problem: halve HBM traffic with bf16 (harness gate is 2e-2,
    bf16 end-to-end lands ~1e-3) and remove every on-device transpose by
    staging x TRANSPOSED on the host: xT [384, B/8] bf16 per core (rows 375..383
    zero-padded so the contraction tiles are a uniform K=128).
  - per core: out.T[75, B/8] = sum_c Wd_c.T @ xT_c with Wd chunk as the PE
    stationary operand (75 cols to load vs 128 for x chunks) and xT streaming
    as the moving operand in 512-col sub-blocks accumulating over the 3 K
    chunks in PSUM. Input DMA is one fully contiguous 3 MB transfer per 4096
    rows; output is written back transposed ([75, B/8] bf16, 8 KB runs) and
    un-transposed on the host.
"""

import numpy as np
import ml_dtypes

BF16 = np.dtype(ml_dtypes.bfloat16)

B = 262144
NCORES = 8
B_CORE = B // NCORES  # 32768
F = 375   # input cols (25 groups * 15)
FP = 384  # padded to 3 chunks of 128
O = 75    # output cols (25 groups * 3)
OUT_DIM = 75
NB = 4096          # batch cols per super-block (one input DMA)
N_SUP = B_CORE // NB  # 8
NSB = 512          # moving-operand free size per matmul
GROUPS = 2         # PSUM groups per super-block
SB_PER_G = NB // (GROUPS * NSB)  # 4 sub-blocks -> 4 PSUM banks per group

_compiled = {}


def _build_bass():
    import concourse.mybir as mybir
    import concourse.tile as tile
    from concourse import bacc

    f32 = mybir.dt.float32
    bf16 = mybir.dt.bfloat16
    nc = bacc.Bacc()
    xt_d = nc.dram_tensor("xt", [FP, B_CORE], bf16, kind="ExternalInput")
    w_d = nc.dram_tensor("wd", [3, 128, O], bf16, kind="ExternalInput")
    ot_d = nc.dram_tensor("ot", [O, B_CORE], bf16, kind="ExternalOutput")

    with tile.TileContext(nc) as tc:
        with (
            tc.tile_pool(name="const", bufs=1) as cpool,
            tc.tile_pool(name="xin", bufs=3) as xpool,
            tc.tile_pool(name="osb", bufs=3) as opool,
            tc.tile_pool(name="acc", bufs=2, space="PSUM") as pacc,
        ):
            wd = cpool.tile([128, 3, O], bf16)
            nc.sync.dma_start(wd[:], w_d[:].rearrange("c k n -> k c n"))

            # PE instructions carry at most one semaphore wait; burn the wd
            # DMA dep with a throwaway matmul so real matmuls only wait on
            # their x DMA.
            warm = pacc.tile([128, SB_PER_G * NSB], f32, tag="acc")
            nc.tensor.matmul(
                warm[:O, :O], wd[:, 0, :], wd[:, 0, :], start=True, stop=True
            )

            for s in range(N_SUP):
                r0 = s * NB
                xin = xpool.tile([128, 3, NB], bf16)
                nc.sync.dma_start(
                    xin[:],
                    xt_d[:, r0 : r0 + NB].rearrange("(c p) n -> p c n", p=128),
                )
                osb = opool.tile([O, GROUPS, NB // GROUPS], bf16)
                for g in range(GROUPS):
                    acc = pacc.tile([128, SB_PER_G * NSB], f32, tag="acc")
                    for c in range(3):
                        for sb in range(SB_PER_G):
                            col0 = g * (NB // GROUPS) + sb * NSB
                            nc.tensor.matmul(
                                acc[:O, sb * NSB : (sb + 1) * NSB],
                                wd[:, c, :],
                                xin[:, c, col0 : col0 + NSB],
                                start=(c == 0),
                                stop=(c == 2),
                            )
                    nc.vector.tensor_copy(osb[:, g, :], acc[:O, :])
                nc.sync.dma_start(
                    ot_d[:, r0 : r0 + NB].rearrange("p (g n) -> p g n", g=GROUPS),
                    osb[:],
                )
    nc.compile()
    return nc


def _get_nc():
    if "nc" not in _compiled:
        _compiled["nc"] = _build_bass()
    return _compiled["nc"]


def _build_wd_chunks(W, k_idx, v_idx):
    """Dense [3, 128, 75] chunked block-diagonal weight from stacked W."""
    Wd = np.zeros((FP, O), dtype=np.float32)
    kk = np.asarray(k_idx)
    vv = np.asarray(v_idx)
    Ww = np.asarray(W)
    # Wd[k_idx[g,i], v_idx[g,j]] = W[g, j, i]
    Wd[kk[:, :, None], vv[:, None, :]] = Ww.transpose(0, 2, 1)
    return np.ascontiguousarray(Wd.reshape(3, 128, O).astype(BF16))


def kernel(x, W, k_idx, v_idx, **_unused):
    from concourse.bass_utils import run_bass_kernel_spmd

    x = np.asarray(x, dtype=np.float32)
    wd3 = _build_wd_chunks(W, k_idx, v_idx)
    nc = _get_nc()

    in_maps = []
    for i in range(NCORES):
        xt = np.zeros((FP, B_CORE), dtype=BF16)
        xt[:F] = x[i * B_CORE : (i + 1) * B_CORE].T.astype(BF16)
        in_maps.append({"xt": xt, "wd": wd3})
    res = run_bass_kernel_spmd(nc, in_maps, list(range(NCORES)))
    parts = [res.results[i]["ot"] for i in range(NCORES)]
    got = np.concatenate(parts, axis=1).T.astype(np.float32)  # [B, 75]

    vflat = np.asarray(v_idx).reshape(-1)
    if vflat.shape[0] == OUT_DIM and np.array_equal(vflat, np.arange(OUT_DIM)):
        return np.ascontiguousarray(got)
    out = np.zeros((x.shape[0], OUT_DIM), dtype=np.float32)
    out[:, vflat] = got
    return out


# revision 7
# speedup vs baseline: 2.5418x; 1.0612x over previous
"""Trainium2 Bass kernel for grouped block-diagonal MLP (gnn_message_passing).

Computation: out[b, 3g+j] = sum_i x[b, 15g+i] * W[g, j, i]   (g<25, i<15, j<3)
Equivalent to out = x @ Wd where Wd is a [375, 75] block-diagonal matrix built
from the 25 stacked [3, 15] Linear weights (scattered per k_idx/v_idx).

Strategy (pure data parallel, 8 cores), v3:
  - memory-regime problem: halve HBM traffic with bf16 (harness gate is 2e-2,
    bf16 end-to-end lands ~3e-3) and remove every on-device transpose by
    staging x TRANSPOSED on the host, laid out so each input DMA is one fully
    contiguous 24 KB run per partition: xt [128, 8 supers, 3 K-chunks, 4096]
    bf16 per core (K rows 375..383 zero-padded for a uniform K=128).
  - per core: out.T[75, B/8] = sum_c Wd_c.T @ xT_c with the Wd chunk as the PE
    stationary operand (75-col LDWEIGHTS) and xT streaming as the moving
    operand in 512-col sub-blocks, accumulating the 3 K-chunks in PSUM
    (4 banks per group, 2 groups in flight). One DVE + one ACT cast move
    each group fp32 PSUM -> bf16 SBUF in parallel halves.
  - input DMAs ride the sync (SP) HWDGE ring; weight + output DMAs ride the
    scalar (ACT) HWDGE ring so writes never FIFO-serialize behind the input
    stream. Output goes back transposed ([75, B/8] bf16) and is un-transposed
    on the host.
"""

import numpy as np
import ml_dtypes

BF16 = np.dtype(ml_dtypes.bfloat16)

B = 262144
NCORES = 8
B_CORE = B // NCORES  # 32768
F = 375   # input cols (25 groups * 15)
FP = 384  # padded to 3 chunks of 128
O = 75    # output cols (25 groups * 3)
OUT_DIM = 75
NB = 4096          # batch cols per super-block (one input DMA)
N_SUP = B_CORE // NB  # 8
NSB = 512          # moving-operand free size per matmul
GROUPS = 2         # PSUM groups per super-block
NG = NB // GROUPS  # 2048 cols per group
SB_PER_G = NG // NSB  # 4 sub-blocks -> 4 PSUM banks per group

_compiled = {}


def _build_bass():
    import concourse.mybir as mybir
    import concourse.tile as tile
    from concourse import bacc

    f32 = mybir.dt.float32
    bf16 = mybir.dt.bfloat16
    nc = bacc.Bacc()
    xt_d = nc.dram_tensor("xt", [128, N_SUP, 3, NB], bf16, kind="ExternalInput")
    w_d = nc.dram_tensor("wd", [3, 128, O], bf16, kind="ExternalInput")
    ot_d = nc.dram_tensor("ot", [O, B_CORE], bf16, kind="ExternalOutput")

    with tile.TileContext(nc) as tc:
        with (
            tc.tile_pool(name="const", bufs=1) as cpool,
            tc.tile_pool(name="xin", bufs=3) as xpool,
            tc.tile_pool(name="osb", bufs=4) as opool,
            tc.tile_pool(name="acc", bufs=2, space="PSUM") as pacc,
        ):
            wd = cpool.tile([128, 3, O], bf16)
            nc.scalar.dma_start(wd[:], w_d[:].rearrange("c k n -> k c n"))

            # PE instructions carry at most one semaphore wait; burn the wd
            # DMA dep with a throwaway matmul so real matmuls only wait on
            # their x DMA.
            warm = pacc.tile([128, SB_PER_G * NSB], f32, tag="acc")
            nc.tensor.matmul(
                warm[:O, :O], wd[:, 0, :], wd[:, 0, :], start=True, stop=True
            )

            for s in range(N_SUP):
                r0 = s * NB
                xin = xpool.tile([128, 3, NB], bf16)
                nc.sync.dma_start(xin[:], xt_d[:, s, :, :])
                for g in range(GROUPS):
                    acc = pacc.tile([128, SB_PER_G * NSB], f32, tag="acc")
                    for c in range(3):
                        for sb in range(SB_PER_G):
                            col0 = g * NG + sb * NSB
                            nc.tensor.matmul(
                                acc[:O, sb * NSB : (sb + 1) * NSB],
                                wd[:, c, :],
                                xin[:, c, col0 : col0 + NSB],
                                start=(c == 0),
                                stop=(c == 2),
                            )
                    osb = opool.tile([O, NG], bf16)
                    half = NG // 2
                    nc.vector.tensor_copy(osb[:, :half], acc[:O, :half])
                    nc.scalar.copy(osb[:, half:], acc[:O, half:])
                    nc.scalar.dma_start(
                        ot_d[:, r0 + g * NG : r0 + (g + 1) * NG], osb[:]
                    )
    nc.compile()
    return nc


def _get_nc():
    if "nc" not in _compiled:
        _compiled["nc"] = _build_bass()
    return _compiled["nc"]


def _build_wd_chunks(W, k_idx, v_idx):
    """Dense [3, 128, 75] chunked block-diagonal weight from stacked W."""
    Wd = np.zeros((FP, O), dtype=np.float32)
    kk = np.asarray(k_idx)
    vv = np.asarray(v_idx)
    Ww = np.asarray(W)
    # Wd[k_idx[g,i], v_idx[g,j]] = W[g, j, i]
    Wd[kk[:, :, None], vv[:, None, :]] = Ww.transpose(0, 2, 1)
    return np.ascontiguousarray(Wd.reshape(3, 128, O).astype(BF16))


def _shard_x(x, i):
    """Core i's input: [128, N_SUP, 3, NB] bf16 with xt[p,s,c,n] =
    x[i*B_CORE + s*NB + n, c*128 + p] (rows >= F are zero padding)."""
    xT = np.zeros((FP, B_CORE), dtype=BF16)
    xT[:F] = x[i * B_CORE : (i + 1) * B_CORE].T.astype(BF16)
    return np.ascontiguousarray(
        xT.reshape(3, 128, N_SUP, NB).transpose(1, 2, 0, 3)
    )


def kernel(x, W, k_idx, v_idx, **_unused):
    from concourse.bass_utils import run_bass_kernel_spmd

    x = np.asarray(x, dtype=np.float32)
    wd3 = _build_wd_chunks(W, k_idx, v_idx)
    nc = _get_nc()

    in_maps = [{"xt": _shard_x(x, i), "wd": wd3} for i in range(NCORES)]
    res = run_bass_kernel_spmd(nc, in_maps, list(range(NCORES)))
    parts = [res.results[i]["ot"] for i in range(NCORES)]
    got = np.concatenate(parts, axis=1).T.astype(np.float32)  # [B, 75]

    vflat = np.asarray(v_idx).reshape(-1)
    if vflat.shape[0] == OUT_DIM and np.array_equal(vflat, np.arange(OUT_DIM)):
        return np.ascontiguousarray(got)
    out = np.zeros((x.shape[0], OUT_DIM), dtype=np.float32)
    out[:, vflat] = got
    return out
